# revision 5
# baseline (speedup 1.0000x reference)
"""Trainium2 Bass kernel for BinaryHead: logits = (l2norm(fea) @ W.T + b) * 16.

Sharding: data-parallel over the batch dim across 8 NeuronCores (2048 rows
each).  The host stages each core's shard TRANSPOSED ([emb, batch]) so the
embedding/contraction dim lands on SBUF partitions, which is what the
TensorEngine contracts over.

v2 pipeline (single-ring, consumption-ordered delivery):
  - All 16 e-panels stream over the ACT HWDGE ring in the exact order the
    compute consumes them, each into its OWN tile (no read-under-write):
    panel 0 as two 512KB halves (early PE start), panels 1-14 whole
    ([128, 2048] bf16 = natural feaT row-slices, 4KB descriptors = max DMA
    rate), panel 15 as four 128KB chunks so the epilogue pipelines into the
    stream tail.
  - Per panel: z.T[c, b] += Wt_chunk.T @ panel (4-col stationary, bf16),
    squares on ACT (even panels) / DVE (odd panels) into fp8, and one fp8
    DoubleRow matmul per pair contracts both panels' squares into sumsq.
  - Epilogue per 512-col chunk: rnorm = Sqrt(reciprocal(ss) * S^2) (DVE
    reciprocal + ACT Sqrt -- the sanctioned accurate rsqrt), class-broadcast
    via a k=1 f32r matmul into a freed sumsq psum bank, z * rnorm on DVE
    straight out of PSUM, bias add on the otherwise-idle GpSimd, out DMA on
    the SP ring.
  - 24 PE warmup matmuls on a memset tile keep the HAM clock-gate at full
    rate before real data lands.
"""

import os
from contextlib import ExitStack

import numpy as np

NUM_CLASS = 4
EMB = 2048
BATCH = 16384
N_CORES = 8
ROWS = BATCH // N_CORES  # 2048 rows per core
S = 16.0

N_PANELS = EMB // 128  # 16 e-panels per core
N_BCHUNK = ROWS // 512  # 4 psum-width chunks of the batch

DTYPE_CFG = "bf16"

_CACHE = {}


def _build_nc():
    import concourse.bacc as bacc
    import concourse.mybir as mybir
    import concourse.tile as tile
    from concourse.hw_specs import get_activation_tables

    f32 = mybir.dt.float32
    f32r = mybir.dt.float32r
    bf16 = mybir.dt.bfloat16
    fp8 = mybir.dt.float8e4
    Square = mybir.ActivationFunctionType.Square
    Sqrt = mybir.ActivationFunctionType.Sqrt

    nc = bacc.Bacc(
        "TRN2",
        target_bir_lowering=False,
        debug=False,
        enable_asserts=False,
        num_devices=N_CORES,
    )

    feaT = nc.dram_tensor("feaT", [EMB, ROWS], bf16, kind="ExternalInput").ap()
    wt = nc.dram_tensor(
        "wt", [128, N_PANELS * NUM_CLASS], bf16, kind="ExternalInput"
    ).ap()
    onesv = nc.dram_tensor("onesv", [128, 2, 16], fp8, kind="ExternalInput").ap()
    sones = nc.dram_tensor("sones", [1, NUM_CLASS], f32r, kind="ExternalInput").ap()
    sbias = nc.dram_tensor("sbias", [NUM_CLASS, 1], f32, kind="ExternalInput").ap()
    outT = nc.dram_tensor("outT", [NUM_CLASS, ROWS], f32, kind="ExternalOutput").ap()

    with tile.TileContext(nc) as tc, ExitStack() as ctx:
        pconst = ctx.enter_context(tc.tile_pool(name="pconst", bufs=1))
        pfull = ctx.enter_context(tc.tile_pool(name="pfull", bufs=1))
        phalf = ctx.enter_context(tc.tile_pool(name="phalf", bufs=1))
        pchunk = ctx.enter_context(tc.tile_pool(name="pchunk", bufs=1))
        psq = ctx.enter_context(tc.tile_pool(name="psq", bufs=3))
        pep = ctx.enter_context(tc.tile_pool(name="pep", bufs=1))
        pz = ctx.enter_context(tc.tile_pool(name="pz", bufs=1, space="PSUM"))
        ps = ctx.enter_context(tc.tile_pool(name="ps", bufs=1, space="PSUM"))

        # ---- stream issue: everything on the ACT ring, consumption order ----
        wt_s = pconst.tile([128, N_PANELS * NUM_CLASS], bf16)
        nc.scalar.dma_start(out=wt_s, in_=wt)

        x0a = phalf.tile([128, 1024], bf16, name="x0a")
        x0b = phalf.tile([128, 1024], bf16, name="x0b")
        nc.scalar.dma_start(out=x0a, in_=feaT[0:128, 0:1024])
        nc.scalar.dma_start(out=x0b, in_=feaT[0:128, 1024:2048])
        xt = [None] * N_PANELS
        for t in range(1, 15):
            xt[t] = pfull.tile([128, ROWS], bf16, name=f"x{t}")
            nc.scalar.dma_start(out=xt[t], in_=feaT[t * 128 : (t + 1) * 128, :])
        x15 = [pchunk.tile([128, 512], bf16, name=f"x15c{j}") for j in range(N_BCHUNK)]
        for j in range(N_BCHUNK):
            nc.scalar.dma_start(
                out=x15[j], in_=feaT[15 * 128 : 16 * 128, j * 512 : (j + 1) * 512]
            )
        # one ACT table set covering Square + Sqrt + Copy; loaded once, after
        # all stream dma issues so it never delays descriptor generation
        sq_id = list(get_activation_tables(nc.m.arch)).index("sqrt_and_others")
        nc.scalar.add_instruction(
            mybir.InstLoadActFuncSet(name=f"I-{nc.next_id()}", act_func_set_id=sq_id)
        )

        # tiny consts ride the otherwise-idle SP ring
        ones_s = pconst.tile([128, 2, 16], fp8)
        nc.sync.dma_start(out=ones_s, in_=onesv)
        sones_s = pconst.tile([1, NUM_CLASS], f32r)
        nc.sync.dma_start(out=sones_s, in_=sones)
        sbias_s = pconst.tile([NUM_CLASS, 1], f32)
        nc.sync.dma_start(out=sbias_s, in_=sbias)

        warm_s = pconst.tile([128, 64], bf16)
        nc.vector.memset(warm_s, 1.0)

        # ---- accumulators ----
        zt_ps = pz.tile([NUM_CLASS, ROWS], f32, tag="zt")
        ss_ps = [
            ps.tile([1, 512], f32, tag="ssrnb", bufs=4, name=f"ss{j}")
            for j in range(N_BCHUNK)
        ]
        rnb = [
            ps.tile([NUM_CLASS, 512], f32, tag="ssrnb", bufs=4, name=f"rnb{j}")
            for j in range(N_BCHUNK)
        ]
        rr_s = pep.tile([1, ROWS], f32)
        rnorm_s = pep.tile([1, ROWS], f32r)
        z_s = pep.tile([NUM_CLASS, ROWS], f32)
        zr_s = pep.tile([NUM_CLASS, ROWS], f32)
        out_s = pep.tile([NUM_CLASS, ROWS], f32)

        # PE warmup on const data (keeps HAM clock at full rate; garbage is
        # killed by the first real z matmul's start=True)
        for _ in range(24):
            nc.tensor.matmul(
                zt_ps[:, 0:64], warm_s[:, 0:4], warm_s, start=True, stop=True
            )

        def z_mm(t, j, mov, start, stop):
            nc.tensor.matmul(
                zt_ps[:, j * 512 : (j + 1) * 512],
                wt_s[:, t * NUM_CLASS : (t + 1) * NUM_CLASS],
                mov,
                start=start,
                stop=stop,
            )

        def ss_mm(k, j, x2):
            # fp8 DoubleRow: one matmul contracts the panel pair (k=256)
            nc.tensor.matmul(
                ss_ps[j],
                ones_s[:, :, 0:1],
                x2[:, :, j * 512 : (j + 1) * 512],
                perf_mode=mybir.MatmulPerfMode.DoubleRow,
                start=(k == 0),
                stop=(k == 7),
            )

        for k in range(8):
            t0, t1 = 2 * k, 2 * k + 1
            x2 = psq.tile([128, 2, ROWS], fp8, tag="x2")
            if k == 0:
                # panel 0 in halves (early start), panel 1 whole
                nc.scalar.activation(
                    out=x2[:, 0, 0:1024], in_=x0a, func=Square, bias=0.0, scale=1.0
                )
                nc.scalar.activation(
                    out=x2[:, 0, 1024:2048], in_=x0b, func=Square, bias=0.0, scale=1.0
                )
                nc.vector.tensor_mul(x2[:, 1, :], xt[1], xt[1])
                for j in range(2):
                    z_mm(0, j, x0a[:, j * 512 : (j + 1) * 512], True, False)
                for j in range(2, 4):
                    z_mm(0, j, x0b[:, (j - 2) * 512 : (j - 1) * 512], True, False)
                for j in range(N_BCHUNK):
                    z_mm(1, j, xt[1][:, j * 512 : (j + 1) * 512], False, False)
                for j in range(N_BCHUNK):
                    ss_mm(0, j, x2)
            elif k < 7:
                # even panel squares on ACT, odd on DVE: they run concurrently
                nc.scalar.activation(
                    out=x2[:, 0, :], in_=xt[t0], func=Square, bias=0.0, scale=1.0
                )
                nc.vector.tensor_mul(x2[:, 1, :], xt[t1], xt[t1])
                for j in range(N_BCHUNK):
                    z_mm(t0, j, xt[t0][:, j * 512 : (j + 1) * 512], False, False)
                for j in range(N_BCHUNK):
                    z_mm(t1, j, xt[t1][:, j * 512 : (j + 1) * 512], False, False)
                for j in range(N_BCHUNK):
                    ss_mm(k, j, x2)
            else:
                # tail pair: panel 14 whole, panel 15 chunked so each chunk's
                # ss stop (and epilogue) fires as soon as that chunk lands
                nc.scalar.activation(
                    out=x2[:, 0, :], in_=xt[14], func=Square, bias=0.0, scale=1.0
                )
                for j in range(N_BCHUNK):
                    z_mm(14, j, xt[14][:, j * 512 : (j + 1) * 512], False, False)
                for j in range(N_BCHUNK):
                    nc.vector.tensor_mul(
                        x2[:, 1, j * 512 : (j + 1) * 512], x15[j], x15[j]
                    )
                    z_mm(15, j, x15[j], False, True)
                    ss_mm(7, j, x2)
                Copy = mybir.ActivationFunctionType.Copy
                for j in range(N_BCHUNK):
                    bsl = slice(j * 512, (j + 1) * 512)
                    # rnorm = sqrt(S^2 / ss): accurate rsqrt = DVE reciprocal
                    # + ACT Sqrt, with the *S scale folded into Sqrt's scale
                    nc.vector.reciprocal(out=rr_s[:, bsl], in_=ss_ps[j])
                    # z leaves PSUM via ACT (DVE can't read two PSUM operands)
                    nc.scalar.activation(
                        out=z_s[:, bsl], in_=zt_ps[:, bsl], func=Copy,
                        bias=0.0, scale=1.0,
                    )
                    nc.scalar.activation(
                        out=rnorm_s[:, bsl],
                        in_=rr_s[:, bsl],
                        func=Sqrt,
                        bias=0.0,
                        scale=float(S * S),
                    )
                    # broadcast across the 4 class partitions via a k=1 f32r
                    # matmul (reuses the freed sumsq psum bank)
                    nc.tensor.matmul(
                        rnb[j], sones_s, rnorm_s[:, bsl], start=True, stop=True
                    )
                    nc.vector.tensor_mul(zr_s[:, bsl], z_s[:, bsl], rnb[j])
                    nc.gpsimd.tensor_scalar_add(
                        out_s[:, bsl], in0=zr_s[:, bsl], scalar1=sbias_s
                    )
                    nc.sync.dma_start(out=outT[:, bsl], in_=out_s[:, bsl])

    nc.compile()
    return nc


def _get_nc():
    if "nc" not in _CACHE:
        _CACHE["nc"] = _build_nc()
    return _CACHE["nc"]


def _stage_inputs(fea, W, b):
    import ml_dtypes

    fea = np.asarray(fea, dtype=np.float32)
    W = np.asarray(W, dtype=np.float32)
    b = np.asarray(b, dtype=np.float32)

    # wt[p, 4t+c] = W[c, 128t+p]
    wt = np.ascontiguousarray(
        W.reshape(NUM_CLASS, N_PANELS, 128).transpose(2, 1, 0).reshape(128, -1)
    ).astype(ml_dtypes.bfloat16)
    onesv = np.zeros((128, 2, 16), dtype=ml_dtypes.float8_e4m3)
    onesv[:, :, 0] = 1.0
    sones = np.ones((1, NUM_CLASS), dtype=np.float32)
    sbias = (S * b).reshape(NUM_CLASS, 1).astype(np.float32)

    in_maps = []
    for i in range(N_CORES):
        shard = fea[i * ROWS : (i + 1) * ROWS, :]
        feaT = np.ascontiguousarray(shard.T).astype(ml_dtypes.bfloat16)
        in_maps.append(
            {"feaT": feaT, "wt": wt, "onesv": onesv, "sones": sones, "sbias": sbias}
        )
    return in_maps


def run(fea, W, b, trace=False):
    from concourse.bass_utils import run_bass_kernel_spmd

    nc = _get_nc()
    in_maps = _stage_inputs(fea, W, b)
    res = run_bass_kernel_spmd(nc, in_maps, core_ids=list(range(N_CORES)), trace=trace)
    out = np.empty((BATCH, NUM_CLASS), dtype=np.float32)
    for i in range(N_CORES):
        out[i * ROWS : (i + 1) * ROWS, :] = res.results[i]["outT"].T
    return out, res


def kernel(fea, W, b):
    out, _ = run(fea, W, b, trace=False)
    return out


# revision 16
# speedup vs baseline: 1.7986x; 1.7986x over previous
"""Trainium2 Bass kernel for BinaryHead: logits = (l2norm(fea) @ W.T + b) * 16.

Sharding: data-parallel over the batch dim across 8 NeuronCores (2048 rows
each).  The host stages each core's shard TRANSPOSED ([emb, batch]) so the
embedding/contraction dim lands on SBUF partitions, which is what the
TensorEngine contracts over.

v3 pipeline:
  - All 16 e-panels stream over the SP HWDGE ring in exact consumption order
    (the ACT engine issues no DMAs, so its compute queue never stalls behind
    ring backpressure).  Each panel lands in its OWN tile (no
    read-under-write): panel 0 as two halves for an early PE start, panels
    1-14 whole ([128, 2048] bf16 = natural feaT row-slices, 4KB descriptors
    = max DMA rate), panel 15 as four 128KB chunks so the epilogue pipelines
    into the stream tail.
  - PSUM layout [16, 512]: partition p = 4*batch_chunk + class.  Each z
    matmul writes partitions 4j..4j+3 of one bank; sumsq rows live on
    partitions j of a second bank (fp8 DoubleRow contracts each panel pair).
    This makes the entire epilogue run as single 16-lane-wide ops instead of
    per-chunk [1,512] ops: Ln+Exp rsqrt on ACT over [4,512], ONE kron(I4,1s)
    k=4 matmul broadcasting rnorm to all 16 partitions, one DVE multiply and
    one DVE bias-add, one output DMA.
  - Squares: even panels on ACT, odd on DVE (concurrent); ss matmuls for
    pair k are issued one pair late so the PE never waits on squares; the
    last pair's squares are chunked so each chunk's ss-stop fires as soon as
    that chunk lands.
  - 24 PE warmup matmuls on a memset tile keep the HAM clock-gate at full
    rate before real data lands.
"""

import os
from contextlib import ExitStack

import numpy as np

NUM_CLASS = 4
EMB = 2048
BATCH = 16384
N_CORES = 8
ROWS = BATCH // N_CORES  # 2048 rows per core
S = 16.0

N_PANELS = EMB // 128  # 16 e-panels per core
N_BCHUNK = ROWS // 512  # 4 psum-width chunks of the batch

DTYPE_CFG = "bf16"

_CACHE = {}


def _build_nc():
    import concourse.bacc as bacc
    import concourse.mybir as mybir
    import concourse.tile as tile

    f32 = mybir.dt.float32
    f32r = mybir.dt.float32r
    bf16 = mybir.dt.bfloat16
    fp8 = mybir.dt.float8e4
    Square = mybir.ActivationFunctionType.Square
    Ln = mybir.ActivationFunctionType.Ln
    Exp = mybir.ActivationFunctionType.Exp
    Copy = mybir.ActivationFunctionType.Copy

    nc = bacc.Bacc(
        "TRN2",
        target_bir_lowering=False,
        debug=False,
        enable_asserts=False,
        num_devices=N_CORES,
    )

    feaT = nc.dram_tensor("feaT", [EMB, ROWS], bf16, kind="ExternalInput").ap()
    # per-(panel, chunk) zero-padded stationaries: W only at cols 4j..4j+3,
    # so every z matmul targets the full [16, 512] psum region at base
    # partition 0 (PE requires base 0/32/64) and rows of other chunks get +0
    wt0 = nc.dram_tensor("wt0", [128, N_BCHUNK * 16], bf16, kind="ExternalInput").ap()
    wtr = nc.dram_tensor(
        "wtr", [128, 15 * N_BCHUNK * 16], bf16, kind="ExternalInput"
    ).ap()
    onesv = nc.dram_tensor("onesv", [128, 2, 16], fp8, kind="ExternalInput").ap()
    bc16 = nc.dram_tensor("bc16", [NUM_CLASS, 16], f32r, kind="ExternalInput").ap()
    sbias = nc.dram_tensor("sbias", [16, 1], f32, kind="ExternalInput").ap()
    outT = nc.dram_tensor("outT", [16, 512], f32, kind="ExternalOutput").ap()

    with tile.TileContext(nc) as tc, ExitStack() as ctx:
        pconst = ctx.enter_context(tc.tile_pool(name="pconst", bufs=1))
        pdata = ctx.enter_context(tc.tile_pool(name="pdata", bufs=1))
        psq = ctx.enter_context(tc.tile_pool(name="psq", bufs=3))
        pep = ctx.enter_context(tc.tile_pool(name="pep", bufs=1))
        pz = ctx.enter_context(tc.tile_pool(name="pz", bufs=1, space="PSUM"))

        # ---- stream issue: everything on the SP ring, consumption order ----
        wt0_s = pconst.tile([128, N_BCHUNK * 16], bf16)
        nc.sync.dma_start(out=wt0_s, in_=wt0)
        ones_s = pconst.tile([128, 2, 16], fp8)
        nc.sync.dma_start(out=ones_s, in_=onesv)
        bc16_s = pconst.tile([NUM_CLASS, 16], f32r)
        nc.sync.dma_start(out=bc16_s, in_=bc16)
        sbias_s = pconst.tile([16, 1], f32)
        nc.sync.dma_start(out=sbias_s, in_=sbias)

        x0a = pdata.tile([128, 1024], bf16, name="x0a")
        x0b = pdata.tile([128, 1024], bf16, name="x0b")
        nc.sync.dma_start(out=x0a, in_=feaT[0:128, 0:1024])
        nc.sync.dma_start(out=x0b, in_=feaT[0:128, 1024:2048])
        # panels 1-15's stationaries stream while panel 0 computes
        wtr_s = pconst.tile([128, 15 * N_BCHUNK * 16], bf16)
        nc.sync.dma_start(out=wtr_s, in_=wtr)
        xt = [None] * N_PANELS
        for t in range(1, 15):
            xt[t] = pdata.tile([128, ROWS], bf16, name=f"x{t}")
            nc.sync.dma_start(out=xt[t], in_=feaT[t * 128 : (t + 1) * 128, :])
        x15 = [pdata.tile([128, 512], bf16, name=f"x15c{j}") for j in range(N_BCHUNK)]
        for j in range(N_BCHUNK):
            nc.sync.dma_start(
                out=x15[j], in_=feaT[15 * 128 : 16 * 128, j * 512 : (j + 1) * 512]
            )

        warm_s = pconst.tile([128, 64], bf16)
        nc.vector.memset(warm_s, 1.0)
        # rsqrt via exp(-0.5*ln(ss) + ln(S)): folds the *S scale in for free
        lnS_s = pconst.tile([NUM_CLASS, 1], f32)
        nc.vector.memset(lnS_s, float(np.log(S)))

        # ---- PSUM: [16, 512] layout, partition p = 4*chunk + class ----
        zt_ps = pz.tile([16, 512], f32, tag="zt")
        ss_ps = pz.tile([NUM_CLASS, 512], f32, tag="ss")
        rnb_ps = pz.tile([16, 512], f32, tag="rnb")

        def wslice(t, j):
            # [128, 16] stationary with W panel t at cols 4j..4j+3, else 0
            if t == 0:
                return wt0_s[:, j * 16 : (j + 1) * 16]
            i = (t - 1) * N_BCHUNK + j
            return wtr_s[:, i * 16 : (i + 1) * 16]

        lnss_s = pep.tile([NUM_CLASS, 512], f32)
        rnorm_s = pep.tile([NUM_CLASS, 512], f32r)
        z_s = pep.tile([16, 512], f32)
        zr_s = pep.tile([16, 512], f32)
        out_s = pep.tile([16, 512], f32)

        # PE warmup on const data (keeps HAM clock at full rate; garbage is
        # killed by the first real matmuls' start=True)
        for _ in range(24):
            nc.tensor.matmul(
                zt_ps[0:4, 0:64], warm_s[:, 0:4], warm_s, start=True, stop=True
            )

        def z_mm(t, j, mov, start, stop):
            nc.tensor.matmul(
                zt_ps,
                wslice(t, j),
                mov,
                start=start,
                stop=stop,
            )

        def ss_mm(k, j, x2):
            # fp8 DoubleRow: one matmul contracts the panel pair (k=256);
            # the stationary window's single ones column (index 5j within
            # onesv) steers chunk j's sum onto psum partition j, +0 elsewhere
            nc.tensor.matmul(
                ss_ps,
                ones_s[:, :, 4 * j : 4 * (j + 1)],
                x2[:, :, j * 512 : (j + 1) * 512],
                perf_mode=mybir.MatmulPerfMode.DoubleRow,
                start=(k == 0 and j == 0),
                stop=(k == 7 and j == N_BCHUNK - 1),
            )

        x2s = []  # per-pair square tiles (psq ring of 3)
        for k in range(8):
            t0, t1 = 2 * k, 2 * k + 1
            x2 = psq.tile([128, 2, ROWS], fp8, tag="x2")
            x2s.append(x2)
            if k == 0:
                nc.scalar.activation(
                    out=x2[:, 0, 0:1024], in_=x0a, func=Square, bias=0.0, scale=1.0
                )
                nc.scalar.activation(
                    out=x2[:, 0, 1024:2048], in_=x0b, func=Square, bias=0.0, scale=1.0
                )
                nc.vector.tensor_mul(x2[:, 1, :], xt[1], xt[1])
                for j in range(2):
                    z_mm(0, j, x0a[:, j * 512 : (j + 1) * 512], j == 0, False)
                for j in range(2, 4):
                    z_mm(0, j, x0b[:, (j - 2) * 512 : (j - 1) * 512], False, False)
                for j in range(N_BCHUNK):
                    z_mm(1, j, xt[1][:, j * 512 : (j + 1) * 512], False, False)
            elif k < 7:
                # even panel squares on ACT, odd on DVE (concurrent engines)
                nc.scalar.activation(
                    out=x2[:, 0, :], in_=xt[t0], func=Square, bias=0.0, scale=1.0
                )
                nc.vector.tensor_mul(x2[:, 1, :], xt[t1], xt[t1])
                for j in range(N_BCHUNK):
                    z_mm(t0, j, xt[t0][:, j * 512 : (j + 1) * 512], False, False)
                for j in range(N_BCHUNK):
                    z_mm(t1, j, xt[t1][:, j * 512 : (j + 1) * 512], False, False)
                # ss for the PREVIOUS pair: its squares finished while this
                # pair streamed, so the PE never waits on ACT/DVE here
                for j in range(N_BCHUNK):
                    ss_mm(k - 1, j, x2s[k - 1])
            else:
                # tail pair: panel 14 squares chunked on ACT, panel 15
                # chunked on DVE, so each chunk's ss(7) fires on arrival
                for j in range(N_BCHUNK):
                    nc.scalar.activation(
                        out=x2[:, 0, j * 512 : (j + 1) * 512],
                        in_=xt[14][:, j * 512 : (j + 1) * 512],
                        func=Square,
                        bias=0.0,
                        scale=1.0,
                    )
                for j in range(N_BCHUNK):
                    z_mm(14, j, xt[14][:, j * 512 : (j + 1) * 512], False, False)
                for j in range(N_BCHUNK):
                    ss_mm(6, j, x2s[6])
                for j in range(N_BCHUNK):
                    nc.vector.tensor_mul(
                        x2[:, 1, j * 512 : (j + 1) * 512], x15[j], x15[j]
                    )
                    z_mm(15, j, x15[j], False, j == N_BCHUNK - 1)
                    ss_mm(7, j, x2)

        # ---- epilogue: single wide ops on the [16, 512] layout ----
        # z leaves PSUM via ACT (frees DVE; runs parallel with Ln/Exp chain)
        nc.scalar.activation(out=z_s, in_=zt_ps, func=Copy, bias=0.0, scale=1.0)
        # rnorm = S/sqrt(ss) via exp(-0.5*ln(ss) + ln(S)) -- one [4,512] op
        # per stage covers all four chunks on four lanes
        nc.scalar.activation(
            out=lnss_s, in_=ss_ps, func=Ln, bias=0.0, scale=1.0
        )
        nc.scalar.activation(
            out=rnorm_s, in_=lnss_s, func=Exp, bias=lnS_s, scale=-0.5
        )
        # ONE k=4 matmul broadcasts rnorm chunk rows to all 16 partitions:
        # stat[k, p] = 1 iff p//4 == k  (kron(I4, ones(1,4)))
        nc.tensor.matmul(rnb_ps, bc16_s, rnorm_s, start=True, stop=True)
        nc.vector.tensor_mul(zr_s, z_s, rnb_ps)
        nc.vector.tensor_scalar_add(out_s, in0=zr_s, scalar1=sbias_s)
        nc.sync.dma_start(out=outT, in_=out_s)

    nc.compile()
    return nc


def _get_nc():
    if "nc" not in _CACHE:
        _CACHE["nc"] = _build_nc()
    return _CACHE["nc"]


def _stage_inputs(fea, W, b):
    import ml_dtypes

    fea = np.asarray(fea, dtype=np.float32)
    W = np.asarray(W, dtype=np.float32)
    b = np.asarray(b, dtype=np.float32)

    # zero-padded per-(panel t, chunk j) stationaries [128, 16]:
    # col 4j+c = W[c, 128t+p], other cols 0
    wtall = np.zeros((N_PANELS, N_BCHUNK, 128, 16), dtype=np.float32)
    for t in range(N_PANELS):
        for j in range(N_BCHUNK):
            wtall[t, j, :, 4 * j : 4 * j + 4] = W[:, t * 128 : (t + 1) * 128].T
    wtall = wtall.transpose(2, 0, 1, 3)  # [128, t, j, 16]
    wt0 = np.ascontiguousarray(wtall[:, 0].reshape(128, -1)).astype(ml_dtypes.bfloat16)
    wtr = np.ascontiguousarray(wtall[:, 1:].reshape(128, -1)).astype(
        ml_dtypes.bfloat16
    )
    # ss stationary windows: within window j (cols 4j..4j+3), only column j
    # (global index 5j) is ones, steering chunk j's sum onto psum partition j
    onesv = np.zeros((128, 2, 16), dtype=ml_dtypes.float8_e4m3)
    for j in range(N_BCHUNK):
        onesv[:, :, 5 * j] = 1.0
    # kron(I4, ones(1,4)): bc16[k, p] = 1 iff p//4 == k
    bc16 = np.kron(np.eye(NUM_CLASS), np.ones((1, NUM_CLASS))).astype(np.float32)
    # sbias[p] = S * b[p % 4]
    sbias = (S * np.tile(b, N_BCHUNK)).reshape(16, 1).astype(np.float32)

    in_maps = []
    for i in range(N_CORES):
        shard = fea[i * ROWS : (i + 1) * ROWS, :]
        feaT = np.ascontiguousarray(shard.T).astype(ml_dtypes.bfloat16)
        in_maps.append(
            {
                "feaT": feaT,
                "wt0": wt0,
                "wtr": wtr,
                "onesv": onesv,
                "bc16": bc16,
                "sbias": sbias,
            }
        )
    return in_maps


def run(fea, W, b, trace=False):
    from concourse.bass_utils import run_bass_kernel_spmd

    nc = _get_nc()
    in_maps = _stage_inputs(fea, W, b)
    res = run_bass_kernel_spmd(nc, in_maps, core_ids=list(range(N_CORES)), trace=trace)
    out = np.empty((BATCH, NUM_CLASS), dtype=np.float32)
    for i in range(N_CORES):
        # outT16[4j + c, b] = out[i*2048 + j*512 + b, c]
        o = res.results[i]["outT"].reshape(N_BCHUNK, NUM_CLASS, 512)
        out[i * ROWS : (i + 1) * ROWS, :] = o.transpose(0, 2, 1).reshape(
            ROWS, NUM_CLASS
        )
    return out, res


def kernel(fea, W, b):
    out, _ = run(fea, W, b, trace=False)
    return out


# revision 21
# speedup vs baseline: 1.9235x; 1.0695x over previous
"""Trainium2 Bass kernel for BinaryHead: logits = (l2norm(fea) @ W.T + b) * 16.

Sharding: data-parallel over the batch dim across 8 NeuronCores (2048 rows
each).  The host stages each core's shard TRANSPOSED ([emb, batch]) so the
embedding/contraction dim lands on SBUF partitions, which is what the
TensorEngine contracts over.

v3 pipeline:
  - All 16 e-panels stream over the SP HWDGE ring in exact consumption order
    (the ACT engine issues no DMAs, so its compute queue never stalls behind
    ring backpressure).  Each panel lands in its OWN tile (no
    read-under-write): panel 0 as two halves for an early PE start, panels
    1-14 whole ([128, 2048] bf16 = natural feaT row-slices, 4KB descriptors
    = max DMA rate), panel 15 as four 128KB chunks so the epilogue pipelines
    into the stream tail.
  - PSUM layout [16, 512]: partition p = 4*batch_chunk + class.  Each z
    matmul writes partitions 4j..4j+3 of one bank; sumsq rows live on
    partitions j of a second bank (fp8 DoubleRow contracts each panel pair).
    This makes the entire epilogue run as single 16-lane-wide ops instead of
    per-chunk [1,512] ops: Ln+Exp rsqrt on ACT over [4,512], ONE kron(I4,1s)
    k=4 matmul broadcasting rnorm to all 16 partitions, one DVE multiply and
    one DVE bias-add, one output DMA.
  - Squares: even panels on ACT, odd on DVE (concurrent); ss matmuls for
    pair k are issued one pair late so the PE never waits on squares; the
    last pair's squares are chunked so each chunk's ss-stop fires as soon as
    that chunk lands.
  - 24 PE warmup matmuls on a memset tile keep the HAM clock-gate at full
    rate before real data lands.
"""

import os
from contextlib import ExitStack

import numpy as np

NUM_CLASS = 4
EMB = 2048
BATCH = 16384
N_CORES = 8
ROWS = BATCH // N_CORES  # 2048 rows per core
S = 16.0

N_PANELS = EMB // 128  # 16 e-panels per core
N_BCHUNK = ROWS // 512  # 4 psum-width chunks of the batch

DTYPE_CFG = "bf16"

_CACHE = {}


def _build_nc():
    import concourse.bacc as bacc
    import concourse.mybir as mybir
    import concourse.tile as tile
    from concourse.hw_specs import get_activation_tables

    f32 = mybir.dt.float32
    f32r = mybir.dt.float32r
    bf16 = mybir.dt.bfloat16
    fp8 = mybir.dt.float8e4
    Square = mybir.ActivationFunctionType.Square
    Ln = mybir.ActivationFunctionType.Ln
    Exp = mybir.ActivationFunctionType.Exp
    Copy = mybir.ActivationFunctionType.Copy

    nc = bacc.Bacc(
        "TRN2",
        target_bir_lowering=False,
        debug=False,
        enable_asserts=False,
        num_devices=N_CORES,
    )

    feaT = nc.dram_tensor("feaT", [EMB, ROWS], bf16, kind="ExternalInput").ap()
    # per-(panel, chunk) zero-padded stationaries: W only at cols 4j..4j+3,
    # so every z matmul targets the full [16, 512] psum region at base
    # partition 0 (PE requires base 0/32/64) and rows of other chunks get +0
    wt0 = nc.dram_tensor("wt0", [128, N_BCHUNK * 16], bf16, kind="ExternalInput").ap()
    wtr = nc.dram_tensor(
        "wtr", [128, 15 * N_BCHUNK * 16], bf16, kind="ExternalInput"
    ).ap()
    onesv = nc.dram_tensor("onesv", [128, 2, 16], fp8, kind="ExternalInput").ap()
    bc16 = nc.dram_tensor("bc16", [NUM_CLASS, 16], f32r, kind="ExternalInput").ap()
    sbias = nc.dram_tensor("sbias", [16, 1], f32, kind="ExternalInput").ap()
    outT = nc.dram_tensor("outT", [16, 512], f32, kind="ExternalOutput").ap()

    with tile.TileContext(nc) as tc, ExitStack() as ctx:
        pconst = ctx.enter_context(tc.tile_pool(name="pconst", bufs=1))
        pdata = ctx.enter_context(tc.tile_pool(name="pdata", bufs=1))
        psq = ctx.enter_context(tc.tile_pool(name="psq", bufs=3))
        pep = ctx.enter_context(tc.tile_pool(name="pep", bufs=1))
        pz = ctx.enter_context(tc.tile_pool(name="pz", bufs=1, space="PSUM"))

        # consts + stationaries ride the ACT ring (5 transfers, fits the ring
        # slots with no engine backpressure); the SP ring is a pure data
        # stream in consumption order starting with the first panel half
        wt0_s = pconst.tile([128, N_BCHUNK * 16], bf16)
        nc.scalar.dma_start(out=wt0_s, in_=wt0)
        ones_s = pconst.tile([128, 2, 16], fp8)
        nc.scalar.dma_start(out=ones_s, in_=onesv)
        bc16_s = pconst.tile([NUM_CLASS, 16], f32r)
        nc.scalar.dma_start(out=bc16_s, in_=bc16)
        sbias_s = pconst.tile([16, 1], f32)
        nc.scalar.dma_start(out=sbias_s, in_=sbias)
        wtr_s = pconst.tile([128, 15 * N_BCHUNK * 16], bf16)
        nc.scalar.dma_start(out=wtr_s, in_=wtr)
        # one ACT table set covering Square+Ln+Exp+Copy, loaded before any
        # activation (after the const dma issues, so those descriptors enter
        # the ring first): the framework's auto-insert pass then sees every
        # activation's func already loaded and emits no further loads
        nlx_id = list(get_activation_tables(nc.m.arch)).index(
            "natural_log_exp_and_others"
        )
        nc.scalar.add_instruction(
            mybir.InstLoadActFuncSet(name=f"I-{nc.next_id()}", act_func_set_id=nlx_id)
        )

        x0a = pdata.tile([128, 1024], bf16, name="x0a")
        x0b = pdata.tile([128, 1024], bf16, name="x0b")
        nc.sync.dma_start(out=x0a, in_=feaT[0:128, 0:1024])
        nc.sync.dma_start(out=x0b, in_=feaT[0:128, 1024:2048])
        xt = [None] * N_PANELS
        for t in range(1, 15):
            xt[t] = pdata.tile([128, ROWS], bf16, name=f"x{t}")
            nc.sync.dma_start(out=xt[t], in_=feaT[t * 128 : (t + 1) * 128, :])
        x15 = [pdata.tile([128, 512], bf16, name=f"x15c{j}") for j in range(N_BCHUNK)]
        for j in range(N_BCHUNK):
            nc.sync.dma_start(
                out=x15[j], in_=feaT[15 * 128 : 16 * 128, j * 512 : (j + 1) * 512]
            )

        warm_s = pconst.tile([128, 64], bf16)
        nc.vector.memset(warm_s, 1.0)
        # rsqrt via exp(-0.5*ln(ss) + ln(S)): folds the *S scale in for free
        lnS_s = pconst.tile([NUM_CLASS, 1], f32)
        nc.vector.memset(lnS_s, float(np.log(S)))

        # ---- PSUM: [16, 512] layout, partition p = 4*chunk + class ----
        zt_ps = pz.tile([16, 512], f32, tag="zt")
        ss_ps = pz.tile([NUM_CLASS, 512], f32, tag="ss")
        rnb_ps = pz.tile([16, 512], f32, tag="rnb")

        def wslice(t, j):
            # [128, 16] stationary with W panel t at cols 4j..4j+3, else 0
            if t == 0:
                return wt0_s[:, j * 16 : (j + 1) * 16]
            i = (t - 1) * N_BCHUNK + j
            return wtr_s[:, i * 16 : (i + 1) * 16]

        lnss_s = pep.tile([NUM_CLASS, 512], f32)
        rnorm_s = pep.tile([NUM_CLASS, 512], f32r)
        z_s = pep.tile([16, 512], f32)
        zr_s = pep.tile([16, 512], f32)
        out_s = pep.tile([16, 512], f32)

        # PE warmup on const data (keeps HAM clock at full rate; garbage is
        # killed by the first real matmuls' start=True)
        for _ in range(24):
            nc.tensor.matmul(
                zt_ps[0:4, 0:64], warm_s[:, 0:4], warm_s, start=True, stop=True
            )

        def z_mm(t, j, mov, start, stop):
            nc.tensor.matmul(
                zt_ps,
                wslice(t, j),
                mov,
                start=start,
                stop=stop,
            )

        def ss_mm(k, j, x2):
            # fp8 DoubleRow: one matmul contracts the panel pair (k=256);
            # the stationary window's single ones column (index 5j within
            # onesv) steers chunk j's sum onto psum partition j, +0 elsewhere
            nc.tensor.matmul(
                ss_ps,
                ones_s[:, :, 4 * j : 4 * (j + 1)],
                x2[:, :, j * 512 : (j + 1) * 512],
                perf_mode=mybir.MatmulPerfMode.DoubleRow,
                start=(k == 0 and j == 0),
                stop=(k == 7 and j == N_BCHUNK - 1),
            )

        x2s = []  # per-pair square tiles (psq ring of 3)
        for k in range(8):
            t0, t1 = 2 * k, 2 * k + 1
            x2 = psq.tile([128, 2, ROWS], fp8, tag="x2")
            x2s.append(x2)
            if k == 0:
                nc.scalar.activation(
                    out=x2[:, 0, 0:1024], in_=x0a, func=Square, bias=0.0, scale=1.0
                )
                nc.scalar.activation(
                    out=x2[:, 0, 1024:2048], in_=x0b, func=Square, bias=0.0, scale=1.0
                )
                nc.vector.tensor_mul(x2[:, 1, :], xt[1], xt[1])
                for j in range(2):
                    z_mm(0, j, x0a[:, j * 512 : (j + 1) * 512], j == 0, False)
                for j in range(2, 4):
                    z_mm(0, j, x0b[:, (j - 2) * 512 : (j - 1) * 512], False, False)
                for j in range(N_BCHUNK):
                    z_mm(1, j, xt[1][:, j * 512 : (j + 1) * 512], False, False)
            elif k < 7:
                # even panel squares on ACT, odd on DVE (concurrent engines)
                nc.scalar.activation(
                    out=x2[:, 0, :], in_=xt[t0], func=Square, bias=0.0, scale=1.0
                )
                nc.vector.tensor_mul(x2[:, 1, :], xt[t1], xt[t1])
                for j in range(N_BCHUNK):
                    z_mm(t0, j, xt[t0][:, j * 512 : (j + 1) * 512], False, False)
                for j in range(N_BCHUNK):
                    z_mm(t1, j, xt[t1][:, j * 512 : (j + 1) * 512], False, False)
                # ss lags the z stream by 1-2 pairs (squares done while later
                # pairs arrive, so the PE never waits on ACT/DVE) and is
                # batched two pairs at a time to halve DoubleRow mode switches
                if k % 2 == 0:
                    for kk in (k - 2, k - 1):
                        for j in range(N_BCHUNK):
                            ss_mm(kk, j, x2s[kk])
            else:
                # tail pair: panel 14 squares chunked on ACT, panel 15
                # chunked on DVE, so each chunk's ss(7) fires on arrival
                for j in range(N_BCHUNK):
                    nc.scalar.activation(
                        out=x2[:, 0, j * 512 : (j + 1) * 512],
                        in_=xt[14][:, j * 512 : (j + 1) * 512],
                        func=Square,
                        bias=0.0,
                        scale=1.0,
                    )
                for j in range(N_BCHUNK):
                    z_mm(14, j, xt[14][:, j * 512 : (j + 1) * 512], False, False)
                for j in range(N_BCHUNK):
                    ss_mm(6, j, x2s[6])
                for j in range(N_BCHUNK):
                    nc.vector.tensor_mul(
                        x2[:, 1, j * 512 : (j + 1) * 512], x15[j], x15[j]
                    )
                    z_mm(15, j, x15[j], False, j == N_BCHUNK - 1)
                    ss_mm(7, j, x2)

        # ---- epilogue: single wide ops on the [16, 512] layout ----
        # z leaves PSUM via ACT (frees DVE; runs parallel with Ln/Exp chain)
        nc.scalar.activation(out=z_s, in_=zt_ps, func=Copy, bias=0.0, scale=1.0)
        # rnorm = S/sqrt(ss) via exp(-0.5*ln(ss) + ln(S)) -- one [4,512] op
        # per stage covers all four chunks on four lanes
        nc.scalar.activation(
            out=lnss_s, in_=ss_ps, func=Ln, bias=0.0, scale=1.0
        )
        nc.scalar.activation(
            out=rnorm_s, in_=lnss_s, func=Exp, bias=lnS_s, scale=-0.5
        )
        # ONE k=4 matmul broadcasts rnorm chunk rows to all 16 partitions:
        # stat[k, p] = 1 iff p//4 == k  (kron(I4, ones(1,4)))
        nc.tensor.matmul(rnb_ps, bc16_s, rnorm_s, start=True, stop=True)
        nc.vector.tensor_mul(zr_s, z_s, rnb_ps)
        nc.vector.tensor_scalar_add(out_s, in0=zr_s, scalar1=sbias_s)
        nc.sync.dma_start(out=outT, in_=out_s)

    nc.compile()
    return nc


def _get_nc():
    if "nc" not in _CACHE:
        _CACHE["nc"] = _build_nc()
    return _CACHE["nc"]


def _stage_inputs(fea, W, b):
    import ml_dtypes

    fea = np.asarray(fea, dtype=np.float32)
    W = np.asarray(W, dtype=np.float32)
    b = np.asarray(b, dtype=np.float32)

    # zero-padded per-(panel t, chunk j) stationaries [128, 16]:
    # col 4j+c = W[c, 128t+p], other cols 0
    wtall = np.zeros((N_PANELS, N_BCHUNK, 128, 16), dtype=np.float32)
    for t in range(N_PANELS):
        for j in range(N_BCHUNK):
            wtall[t, j, :, 4 * j : 4 * j + 4] = W[:, t * 128 : (t + 1) * 128].T
    wtall = wtall.transpose(2, 0, 1, 3)  # [128, t, j, 16]
    wt0 = np.ascontiguousarray(wtall[:, 0].reshape(128, -1)).astype(ml_dtypes.bfloat16)
    wtr = np.ascontiguousarray(wtall[:, 1:].reshape(128, -1)).astype(
        ml_dtypes.bfloat16
    )
    # ss stationary windows: within window j (cols 4j..4j+3), only column j
    # (global index 5j) is ones, steering chunk j's sum onto psum partition j
    onesv = np.zeros((128, 2, 16), dtype=ml_dtypes.float8_e4m3)
    for j in range(N_BCHUNK):
        onesv[:, :, 5 * j] = 1.0
    # kron(I4, ones(1,4)): bc16[k, p] = 1 iff p//4 == k
    bc16 = np.kron(np.eye(NUM_CLASS), np.ones((1, NUM_CLASS))).astype(np.float32)
    # sbias[p] = S * b[p % 4]
    sbias = (S * np.tile(b, N_BCHUNK)).reshape(16, 1).astype(np.float32)

    in_maps = []
    for i in range(N_CORES):
        shard = fea[i * ROWS : (i + 1) * ROWS, :]
        feaT = np.ascontiguousarray(shard.T).astype(ml_dtypes.bfloat16)
        in_maps.append(
            {
                "feaT": feaT,
                "wt0": wt0,
                "wtr": wtr,
                "onesv": onesv,
                "bc16": bc16,
                "sbias": sbias,
            }
        )
    return in_maps


def run(fea, W, b, trace=False):
    from concourse.bass_utils import run_bass_kernel_spmd

    nc = _get_nc()
    in_maps = _stage_inputs(fea, W, b)
    res = run_bass_kernel_spmd(nc, in_maps, core_ids=list(range(N_CORES)), trace=trace)
    out = np.empty((BATCH, NUM_CLASS), dtype=np.float32)
    for i in range(N_CORES):
        # outT16[4j + c, b] = out[i*2048 + j*512 + b, c]
        o = res.results[i]["outT"].reshape(N_BCHUNK, NUM_CLASS, 512)
        out[i * ROWS : (i + 1) * ROWS, :] = o.transpose(0, 2, 1).reshape(
            ROWS, NUM_CLASS
        )
    return out, res


def kernel(fea, W, b):
    out, _ = run(fea, W, b, trace=False)
    return out


# revision 24
# speedup vs baseline: 1.9548x; 1.0163x over previous
"""Trainium2 Bass kernel for BinaryHead: logits = (l2norm(fea) @ W.T + b) * 16.

Sharding: data-parallel over the batch dim across 8 NeuronCores (2048 rows
each).  The host stages each core's shard TRANSPOSED ([emb, batch]) so the
embedding/contraction dim lands on SBUF partitions, which is what the
TensorEngine contracts over.

v3 pipeline:
  - All 16 e-panels stream over the SP HWDGE ring in exact consumption order
    (the ACT engine issues no DMAs, so its compute queue never stalls behind
    ring backpressure).  Each panel lands in its OWN tile (no
    read-under-write): panel 0 as two halves for an early PE start, panels
    1-14 whole ([128, 2048] bf16 = natural feaT row-slices, 4KB descriptors
    = max DMA rate), panel 15 as four 128KB chunks so the epilogue pipelines
    into the stream tail.
  - PSUM layout [16, 512]: partition p = 4*batch_chunk + class.  Each z
    matmul writes partitions 4j..4j+3 of one bank; sumsq rows live on
    partitions j of a second bank (fp8 DoubleRow contracts each panel pair).
    This makes the entire epilogue run as single 16-lane-wide ops instead of
    per-chunk [1,512] ops: Ln+Exp rsqrt on ACT over [4,512], ONE kron(I4,1s)
    k=4 matmul broadcasting rnorm to all 16 partitions, one DVE multiply and
    one DVE bias-add, one output DMA.
  - Squares: even panels on ACT, odd on DVE (concurrent); ss matmuls for
    pair k are issued one pair late so the PE never waits on squares; the
    last pair's squares are chunked so each chunk's ss-stop fires as soon as
    that chunk lands.
  - 24 PE warmup matmuls on a memset tile keep the HAM clock-gate at full
    rate before real data lands.
"""

import os
from contextlib import ExitStack

import numpy as np

NUM_CLASS = 4
EMB = 2048
BATCH = 16384
N_CORES = 8
ROWS = BATCH // N_CORES  # 2048 rows per core
S = 16.0

N_PANELS = EMB // 128  # 16 e-panels per core
N_BCHUNK = ROWS // 512  # 4 psum-width chunks of the batch

DTYPE_CFG = "bf16"

_CACHE = {}


def _build_nc():
    import concourse.bacc as bacc
    import concourse.mybir as mybir
    import concourse.tile as tile
    from concourse.hw_specs import get_activation_tables

    f32 = mybir.dt.float32
    f32r = mybir.dt.float32r
    bf16 = mybir.dt.bfloat16
    fp8 = mybir.dt.float8e4
    Square = mybir.ActivationFunctionType.Square
    Ln = mybir.ActivationFunctionType.Ln
    Exp = mybir.ActivationFunctionType.Exp
    Copy = mybir.ActivationFunctionType.Copy

    nc = bacc.Bacc(
        "TRN2",
        target_bir_lowering=False,
        debug=False,
        enable_asserts=False,
        num_devices=N_CORES,
    )

    feaT = nc.dram_tensor("feaT", [EMB, ROWS], bf16, kind="ExternalInput").ap()
    # per-(panel, chunk) zero-padded stationaries: W only at cols 4j..4j+3,
    # so every z matmul targets the full [16, 512] psum region at base
    # partition 0 (PE requires base 0/32/64) and rows of other chunks get +0
    wt0 = nc.dram_tensor("wt0", [128, N_BCHUNK * 16], bf16, kind="ExternalInput").ap()
    wtr = nc.dram_tensor(
        "wtr", [128, 15 * N_BCHUNK * 16], bf16, kind="ExternalInput"
    ).ap()
    onesv = nc.dram_tensor("onesv", [128, 2, 16], fp8, kind="ExternalInput").ap()
    bc16 = nc.dram_tensor("bc16", [NUM_CLASS, 16], f32r, kind="ExternalInput").ap()
    sbias = nc.dram_tensor("sbias", [16, 1], f32, kind="ExternalInput").ap()
    outT = nc.dram_tensor("outT", [16, 512], f32, kind="ExternalOutput").ap()

    with tile.TileContext(nc) as tc, ExitStack() as ctx:
        pconst = ctx.enter_context(tc.tile_pool(name="pconst", bufs=1))
        pdata = ctx.enter_context(tc.tile_pool(name="pdata", bufs=1))
        psq = ctx.enter_context(tc.tile_pool(name="psq", bufs=3))
        pep = ctx.enter_context(tc.tile_pool(name="pep", bufs=1))
        pz = ctx.enter_context(tc.tile_pool(name="pz", bufs=1, space="PSUM"))

        # one ACT table set covering Square+Ln+Exp+Copy, loaded as the FIRST
        # ACT instruction: the framework's auto-insert pass then sees every
        # activation's func already loaded and emits no further loads, and
        # the load runs during the DGE spin-up instead of the compute phase
        nlx_id = list(get_activation_tables(nc.m.arch)).index(
            "natural_log_exp_and_others"
        )
        nc.scalar.add_instruction(
            mybir.InstLoadActFuncSet(name=f"I-{nc.next_id()}", act_func_set_id=nlx_id)
        )

        # wt0 leads the SP data stream (first z matmuls need it with x0a);
        # the other consts + stationaries ride the ACT ring after the table
        # load, so the SP ring stays a pure consumption-ordered data stream
        wt0_s = pconst.tile([128, N_BCHUNK * 16], bf16)
        nc.sync.dma_start(out=wt0_s, in_=wt0)
        ones_s = pconst.tile([128, 2, 16], fp8)
        nc.scalar.dma_start(out=ones_s, in_=onesv)
        bc16_s = pconst.tile([NUM_CLASS, 16], f32r)
        nc.scalar.dma_start(out=bc16_s, in_=bc16)
        sbias_s = pconst.tile([16, 1], f32)
        nc.scalar.dma_start(out=sbias_s, in_=sbias)
        wtr_s = pconst.tile([128, 15 * N_BCHUNK * 16], bf16)
        nc.scalar.dma_start(out=wtr_s, in_=wtr)

        x0a = pdata.tile([128, 1024], bf16, name="x0a")
        x0b = pdata.tile([128, 1024], bf16, name="x0b")
        nc.sync.dma_start(out=x0a, in_=feaT[0:128, 0:1024])
        nc.sync.dma_start(out=x0b, in_=feaT[0:128, 1024:2048])
        xt = [None] * N_PANELS
        for t in range(1, 15):
            xt[t] = pdata.tile([128, ROWS], bf16, name=f"x{t}")
            nc.sync.dma_start(out=xt[t], in_=feaT[t * 128 : (t + 1) * 128, :])
        x15 = [pdata.tile([128, 512], bf16, name=f"x15c{j}") for j in range(N_BCHUNK)]
        for j in range(N_BCHUNK):
            nc.sync.dma_start(
                out=x15[j], in_=feaT[15 * 128 : 16 * 128, j * 512 : (j + 1) * 512]
            )

        warm_s = pconst.tile([128, 64], bf16)
        nc.vector.memset(warm_s, 1.0)
        # rsqrt via exp(-0.5*ln(ss) + ln(S)): folds the *S scale in for free
        lnS_s = pconst.tile([NUM_CLASS, 1], f32)
        nc.vector.memset(lnS_s, float(np.log(S)))

        # ---- PSUM: [16, 512] layout, partition p = 4*chunk + class ----
        zt_ps = pz.tile([16, 512], f32, tag="zt")
        ss_ps = pz.tile([NUM_CLASS, 512], f32, tag="ss")
        rnb_ps = pz.tile([16, 512], f32, tag="rnb")

        def wslice(t, j):
            # [128, 16] stationary with W panel t at cols 4j..4j+3, else 0
            if t == 0:
                return wt0_s[:, j * 16 : (j + 1) * 16]
            i = (t - 1) * N_BCHUNK + j
            return wtr_s[:, i * 16 : (i + 1) * 16]

        lnss_s = pep.tile([NUM_CLASS, 512], f32)
        rnorm_s = pep.tile([NUM_CLASS, 512], f32r)
        z_s = pep.tile([16, 512], f32)
        zr_s = pep.tile([16, 512], f32)
        out_s = pep.tile([16, 512], f32)

        # PE warmup on const data: the HAM clock-gate only unthrottles after
        # ~3.4us of SUSTAINED PE activity, so burn ~3.5us of dummy matmuls
        # between engine start and first-data arrival (garbage is killed by
        # the first real matmuls' start=True)
        for _ in range(44):
            nc.tensor.matmul(
                zt_ps[0:4, 0:64], warm_s[:, 0:4], warm_s, start=True, stop=True
            )

        def z_mm(t, j, mov, start, stop):
            nc.tensor.matmul(
                zt_ps,
                wslice(t, j),
                mov,
                start=start,
                stop=stop,
            )

        def ss_mm(k, j, x2):
            # fp8 DoubleRow: one matmul contracts the panel pair (k=256);
            # the stationary window's single ones column (index 5j within
            # onesv) steers chunk j's sum onto psum partition j, +0 elsewhere
            nc.tensor.matmul(
                ss_ps,
                ones_s[:, :, 4 * j : 4 * (j + 1)],
                x2[:, :, j * 512 : (j + 1) * 512],
                perf_mode=mybir.MatmulPerfMode.DoubleRow,
                start=(k == 0 and j == 0),
                stop=(k == 7 and j == N_BCHUNK - 1),
            )

        x2s = []  # per-pair square tiles (psq ring of 3)
        for k in range(8):
            t0, t1 = 2 * k, 2 * k + 1
            x2 = psq.tile([128, 2, ROWS], fp8, tag="x2")
            x2s.append(x2)
            if k == 0:
                nc.scalar.activation(
                    out=x2[:, 0, 0:1024], in_=x0a, func=Square, bias=0.0, scale=1.0
                )
                nc.scalar.activation(
                    out=x2[:, 0, 1024:2048], in_=x0b, func=Square, bias=0.0, scale=1.0
                )
                nc.vector.tensor_mul(x2[:, 1, :], xt[1], xt[1])
                for j in range(2):
                    z_mm(0, j, x0a[:, j * 512 : (j + 1) * 512], j == 0, False)
                for j in range(2, 4):
                    z_mm(0, j, x0b[:, (j - 2) * 512 : (j - 1) * 512], False, False)
                for j in range(N_BCHUNK):
                    z_mm(1, j, xt[1][:, j * 512 : (j + 1) * 512], False, False)
            elif k < 7:
                # even panel squares on ACT, odd on DVE (concurrent engines)
                nc.scalar.activation(
                    out=x2[:, 0, :], in_=xt[t0], func=Square, bias=0.0, scale=1.0
                )
                nc.vector.tensor_mul(x2[:, 1, :], xt[t1], xt[t1])
                # ss lags the z stream by 1-2 pairs (squares done while later
                # pairs arrive), batched two pairs at a time to halve
                # DoubleRow mode switches, and issued BEFORE this pair's z so
                # the in-order PE fills its data-wait with ready ss work
                if k % 2 == 0:
                    for kk in (k - 2, k - 1):
                        for j in range(N_BCHUNK):
                            ss_mm(kk, j, x2s[kk])
                for j in range(N_BCHUNK):
                    z_mm(t0, j, xt[t0][:, j * 512 : (j + 1) * 512], False, False)
                for j in range(N_BCHUNK):
                    z_mm(t1, j, xt[t1][:, j * 512 : (j + 1) * 512], False, False)
            else:
                # tail pair: panel 14 squares chunked on ACT, panel 15
                # chunked on DVE, so each chunk's ss(7) fires on arrival
                for j in range(N_BCHUNK):
                    nc.scalar.activation(
                        out=x2[:, 0, j * 512 : (j + 1) * 512],
                        in_=xt[14][:, j * 512 : (j + 1) * 512],
                        func=Square,
                        bias=0.0,
                        scale=1.0,
                    )
                for j in range(N_BCHUNK):
                    z_mm(14, j, xt[14][:, j * 512 : (j + 1) * 512], False, False)
                for j in range(N_BCHUNK):
                    ss_mm(6, j, x2s[6])
                for j in range(N_BCHUNK):
                    nc.vector.tensor_mul(
                        x2[:, 1, j * 512 : (j + 1) * 512], x15[j], x15[j]
                    )
                    z_mm(15, j, x15[j], False, j == N_BCHUNK - 1)
                    ss_mm(7, j, x2)

        # ---- epilogue: single wide ops on the [16, 512] layout ----
        # z leaves PSUM via ACT (frees DVE; runs parallel with Ln/Exp chain)
        nc.scalar.activation(out=z_s, in_=zt_ps, func=Copy, bias=0.0, scale=1.0)
        # rnorm = S/sqrt(ss) via exp(-0.5*ln(ss) + ln(S)) -- one [4,512] op
        # per stage covers all four chunks on four lanes
        nc.scalar.activation(
            out=lnss_s, in_=ss_ps, func=Ln, bias=0.0, scale=1.0
        )
        nc.scalar.activation(
            out=rnorm_s, in_=lnss_s, func=Exp, bias=lnS_s, scale=-0.5
        )
        # ONE k=4 matmul broadcasts rnorm chunk rows to all 16 partitions:
        # stat[k, p] = 1 iff p//4 == k  (kron(I4, ones(1,4)))
        nc.tensor.matmul(rnb_ps, bc16_s, rnorm_s, start=True, stop=True)
        nc.vector.tensor_mul(zr_s, z_s, rnb_ps)
        nc.vector.tensor_scalar_add(out_s, in0=zr_s, scalar1=sbias_s)
        nc.sync.dma_start(out=outT, in_=out_s)

    nc.compile()
    return nc


def _get_nc():
    if "nc" not in _CACHE:
        _CACHE["nc"] = _build_nc()
    return _CACHE["nc"]


def _stage_inputs(fea, W, b):
    import ml_dtypes

    fea = np.asarray(fea, dtype=np.float32)
    W = np.asarray(W, dtype=np.float32)
    b = np.asarray(b, dtype=np.float32)

    # zero-padded per-(panel t, chunk j) stationaries [128, 16]:
    # col 4j+c = W[c, 128t+p], other cols 0
    wtall = np.zeros((N_PANELS, N_BCHUNK, 128, 16), dtype=np.float32)
    for t in range(N_PANELS):
        for j in range(N_BCHUNK):
            wtall[t, j, :, 4 * j : 4 * j + 4] = W[:, t * 128 : (t + 1) * 128].T
    wtall = wtall.transpose(2, 0, 1, 3)  # [128, t, j, 16]
    wt0 = np.ascontiguousarray(wtall[:, 0].reshape(128, -1)).astype(ml_dtypes.bfloat16)
    wtr = np.ascontiguousarray(wtall[:, 1:].reshape(128, -1)).astype(
        ml_dtypes.bfloat16
    )
    # ss stationary windows: within window j (cols 4j..4j+3), only column j
    # (global index 5j) is ones, steering chunk j's sum onto psum partition j
    onesv = np.zeros((128, 2, 16), dtype=ml_dtypes.float8_e4m3)
    for j in range(N_BCHUNK):
        onesv[:, :, 5 * j] = 1.0
    # kron(I4, ones(1,4)): bc16[k, p] = 1 iff p//4 == k
    bc16 = np.kron(np.eye(NUM_CLASS), np.ones((1, NUM_CLASS))).astype(np.float32)
    # sbias[p] = S * b[p % 4]
    sbias = (S * np.tile(b, N_BCHUNK)).reshape(16, 1).astype(np.float32)

    in_maps = []
    for i in range(N_CORES):
        shard = fea[i * ROWS : (i + 1) * ROWS, :]
        feaT = np.ascontiguousarray(shard.T).astype(ml_dtypes.bfloat16)
        in_maps.append(
            {
                "feaT": feaT,
                "wt0": wt0,
                "wtr": wtr,
                "onesv": onesv,
                "bc16": bc16,
                "sbias": sbias,
            }
        )
    return in_maps


def run(fea, W, b, trace=False):
    from concourse.bass_utils import run_bass_kernel_spmd

    nc = _get_nc()
    in_maps = _stage_inputs(fea, W, b)
    res = run_bass_kernel_spmd(nc, in_maps, core_ids=list(range(N_CORES)), trace=trace)
    out = np.empty((BATCH, NUM_CLASS), dtype=np.float32)
    for i in range(N_CORES):
        # outT16[4j + c, b] = out[i*2048 + j*512 + b, c]
        o = res.results[i]["outT"].reshape(N_BCHUNK, NUM_CLASS, 512)
        out[i * ROWS : (i + 1) * ROWS, :] = o.transpose(0, 2, 1).reshape(
            ROWS, NUM_CLASS
        )
    return out, res


def kernel(fea, W, b):
    out, _ = run(fea, W, b, trace=False)
    return out
